# revision 28
# baseline (speedup 1.0000x reference)
"""Trainium2 Bass kernel for nn_CausalWanModel (frame-block-causal attention).

Self-contained: hardcodes shapes from the problem spec.
  B=1, T=3120, D=1536, H=12 heads, hd=128, frame_seqlen=780, 8 cores.

Sharding: sequence-parallel. Core c owns tokens [390c, 390c+390).
Per-core pipeline (emission order == engine queue order):
  k-proj (rope rotation fused; partition swap via a PE permutation matmul)
  -> rmsnorm scales -> k payload writes -> AllGather k half 0 trigger
  -> v-proj groups 0,1 -> AllGather v half 0 -> AllGather k half 1
  -> v group 2 -> AllGather v half 1 -> q-proj (overlaps the collectives).
  Attention per head over 8 rank-blocks of 3 full 128-key tiles plus one
  48-key remainder tile; block-causal mask as per-rank additive exp bias.
  K/V land in SBUF in 3-head quarters via dedicated pools (gpsimd/scalar
  queues) so loads start the moment each collective completes. Softmax
  denominators accumulate in bf16 on the Vector engine, ones-matmul total,
  reciprocal_approx_fast; head finalization is emitted after the next
  head's first QK groups so the PE never stalls on it.
  o-proj (column group 0 weights prefetched during attention).
"""

import math

import numpy as np
import ml_dtypes

import concourse.bacc as bacc
import concourse.mybir as mybir
import concourse.tile as tile
from concourse.bass_utils import run_bass_kernel_spmd

F32 = mybir.dt.float32
BF16 = mybir.dt.bfloat16

NC = 8
T = 3120
D = 1536
H = 12
HD = 128
L = 780  # frame_seqlen
CHUNK = T // NC  # 390 tokens per core
KC = D // 128  # 12 contraction chunks
EPS = 1e-6
SCALE = 1.0 / math.sqrt(HD)
NEG = -30000.0

FULL = 384            # 3 full key tiles per rank; 6 remainder keys
HH = 6                # heads per collective half
QH = 3                # heads per SBUF load quarter
KHALF = 128 * HH * CHUNK       # k payload elems per half, layout [p, h, t]
VMAIN = 128 * 3 * (HH * 128)   # v payload main part, layout [p, j, hc]
VHALF = VMAIN + 6 * (HH * 128)  # + remainder rows tail [t, hc]
TOKSUBS = ((0, 128), (128, 128), (256, 128), (384, 6))


def build_kernel(debug=False):
    nc = bacc.Bacc("TRN2", target_bir_lowering=False, debug=False, num_devices=NC)

    # ---- I/O ----
    xT = nc.dram_tensor("xT", [D, CHUNK], BF16, kind="ExternalInput")
    wq = nc.dram_tensor("wq", [KC, 128, KC * 128], BF16, kind="ExternalInput")
    wk = nc.dram_tensor("wk", [KC, 128, KC * 128], BF16, kind="ExternalInput")
    wv = nc.dram_tensor("wv", [3, KC, 128, 512], BF16, kind="ExternalInput")
    wo = nc.dram_tensor("wo", [KC, 3, 128, 512], BF16, kind="ExternalInput")
    permat = nc.dram_tensor("permat", [128, 128], BF16, kind="ExternalInput")
    cost = nc.dram_tensor("cost", [128, CHUNK], BF16, kind="ExternalInput")
    sint = nc.dram_tensor("sint", [128, CHUNK], BF16, kind="ExternalInput")
    maskv = nc.dram_tensor("maskv", [128, NC], F32, kind="ExternalInput")
    maskr = nc.dram_tensor("maskr", [128, 1], F32, kind="ExternalInput")
    out_part = nc.dram_tensor("out_part", [CHUNK, D], F32, kind="ExternalOutput")

    # ---- collective buffers ----
    k_in = [nc.dram_tensor(f"k_in{g}", [KHALF], BF16) for g in range(2)]
    v_in = [nc.dram_tensor(f"v_in{g}", [VHALF], BF16) for g in range(2)]
    k_out = [nc.dram_tensor(f"k_out{g}", [NC, KHALF], BF16, addr_space="Shared")
             for g in range(2)]
    v_out = [nc.dram_tensor(f"v_out{g}", [NC, VHALF], BF16, addr_space="Shared")
             for g in range(2)]

    with tile.TileContext(nc) as tc:
        with tc.tile_pool(name="const", bufs=1) as cpool, \
             tc.tile_pool(name="a_k", bufs=2) as akp, \
             tc.tile_pool(name="a_v", bufs=2) as avp, \
             tc.tile_pool(name="a_kr", bufs=2) as akrp, \
             tc.tile_pool(name="a_vr", bufs=2) as avrp, \
             tc.tile_pool(name="a_pr", bufs=6) as app, \
             tc.tile_pool(name="a_sb", bufs=3) as asb, \
             tc.tile_pool(name="a_sum", bufs=2) as asum, \
             tc.tile_pool(name="p4w", bufs=1) as p4w:
            qT_sb = cpool.tile([128, KC * CHUNK], BF16, tag="qT_sb")
            attnT_sb = cpool.tile([128, KC * CHUNK], BF16, tag="attnT_sb")
            cost_sb = cpool.tile([128, CHUNK], BF16, tag="cost_sb")
            sint_sb = cpool.tile([128, CHUNK], BF16, tag="sint_sb")
            masks_sb = cpool.tile([128, NC], F32, tag="masks_sb")
            maskr_sb = cpool.tile([128, 1], F32, tag="maskr_sb")
            perm_sb = cpool.tile([128, 128], BF16, tag="perm_sb")
            ones_bf = cpool.tile([128, 1], BF16, tag="ones_bf")
            ones128 = cpool.tile([1, 128], BF16, tag="ones128")
            eps_sb = cpool.tile([1, 1], F32, tag="eps_sb")
            sq_bc = cpool.tile([128, CHUNK], BF16, tag="sq_bc")
            sk_bc = cpool.tile([128, CHUNK], BF16, tag="sk_bc")

            nc.gpsimd.memset(ones_bf[:, :], 1.0)
            nc.gpsimd.memset(ones128[:, :], 1.0)
            nc.gpsimd.memset(eps_sb[:, :], EPS)
            nc.scalar.dma_start(out=cost_sb[:, :], in_=cost[:, :])
            nc.scalar.dma_start(out=sint_sb[:, :], in_=sint[:, :])
            nc.scalar.dma_start(out=masks_sb[:, :], in_=maskv[:, :])
            nc.scalar.dma_start(out=maskr_sb[:, :], in_=maskr[:, :])
            nc.scalar.dma_start(out=perm_sb[:, :], in_=permat[:, :])

            wo_pre = p4w.tile([128, KC * 512], BF16, tag="wo_pre")

            # ===== Phase 1: projections + rmsnorm + rope =====
            with tc.tile_pool(name="p1sb", bufs=3) as p1sb, \
                 tc.tile_pool(name="p1w", bufs=3) as p1w, \
                 tc.tile_pool(name="p1wv", bufs=12) as p1wv, \
                 tc.tile_pool(name="upool", bufs=1) as upool, \
                 tc.tile_pool(name="xpool", bufs=1) as xpool, \
                 tc.tile_pool(name="scl", bufs=2) as sclp, \
                 tc.tile_pool(name="qk_ps", bufs=2, space="PSUM") as qkps, \
                 tc.tile_pool(name="usw_ps", bufs=2, space="PSUM") as uswps, \
                 tc.tile_pool(name="v_ps", bufs=2, space="PSUM") as vps, \
                 tc.tile_pool(name="ssq_ps", bufs=1, space="PSUM") as ssqps:

                xT_sb = xpool.tile([128, KC * CHUNK], BF16, tag="xT_sb")
                for d in range(KC):
                    nc.scalar.dma_start(out=xT_sb[:, d * CHUNK:(d + 1) * CHUNK],
                                        in_=xT[d * 128:(d + 1) * 128, :])

                u_tiles = {(n, d): upool.tile([128, CHUNK], BF16,
                                              name=f"u_{n}_{d}", tag=f"u_{n}_{d}")
                           for n in ("q", "k") for d in range(KC)}
                ssq_ps = {}

                def qk_proj(name, w):
                    # projection + rmsnorm squares + rope rotation.
                    # ssq/perm matmuls for chunk d are emitted after chunk
                    # d+1's projection so the PE never waits on ACT/DVE.
                    ssq_ps[name] = ssqps.tile([1, CHUNK], F32, name=f"ssq_{name}",
                                              tag=f"ssq_{name}")
                    pend = []

                    def tail(d):
                        sq, usw = pend.pop(0)
                        nc.tensor.matmul(ssq_ps[name][:, :], ones_bf[:, :],
                                         sq[:, :],
                                         start=(d == 0), stop=(d == KC - 1))
                        ur = u_tiles[(name, d)]
                        nc.tensor.matmul(usw[:, :], perm_sb[:, :], ur[:, :],
                                         start=True, stop=True)
                        t1 = p1sb.tile([128, CHUNK], BF16, tag="rope_t1",
                                       name="rope_t1")
                        t2 = p1sb.tile([128, CHUNK], BF16, tag="rope_t2",
                                       name="rope_t2")
                        nc.vector.tensor_tensor(t1[:, :], ur[:, :], cost_sb[:, :],
                                                mybir.AluOpType.mult)
                        nc.vector.tensor_tensor(t2[:, :], usw[:, :], sint_sb[:, :],
                                                mybir.AluOpType.mult)
                        nc.vector.tensor_tensor(ur[:, :], t1[:, :], t2[:, :],
                                                mybir.AluOpType.add)

                    for d in range(KC):
                        wt = p1w.tile([128, D], BF16, tag="wqk_t", name="wqk_t")
                        nc.sync.dma_start(out=wt[:, :], in_=w[d, :, :])
                        ps = qkps.tile([128, CHUNK], F32, tag="proj_ps",
                                       name="proj_ps")
                        for c in range(KC):
                            nc.tensor.matmul(
                                ps[:, :],
                                wt[:, c * 128:(c + 1) * 128],
                                xT_sb[:, c * CHUNK:(c + 1) * CHUNK],
                                start=(c == 0), stop=(c == KC - 1))
                        ur = u_tiles[(name, d)]
                        nc.vector.tensor_copy(ur[:, :], ps[:, :])
                        sq = p1sb.tile([128, CHUNK], BF16, tag="sqsb", name="sqsb")
                        nc.scalar.activation(sq[:, :], ps[:, :],
                                             mybir.ActivationFunctionType.Square)
                        usw = uswps.tile([128, CHUNK], F32, tag="usw_ps",
                                         name="usw_ps")
                        pend.append((sq, usw))
                        if d > 0:
                            tail(d - 1)
                    tail(KC - 1)

                def qk_scales(name, sbc):
                    stile = sclp.tile([1, CHUNK], F32, tag="stile", name="stile")
                    rec = sclp.tile([1, CHUNK], F32, tag="rec", name="rec")
                    recb = sclp.tile([1, CHUNK], BF16, tag="recb16", name="recb16")
                    nc.scalar.activation(stile[:, :], ssq_ps[name][:, :],
                                         mybir.ActivationFunctionType.Sqrt,
                                         bias=eps_sb[:, :], scale=1.0 / D)
                    nc.vector.reciprocal_approx_fast(rec[:, :], stile[:, :])
                    nc.vector.tensor_copy(recb[:, :], rec[:, :])
                    # broadcast to all partitions via PE outer product
                    sbc_ps = uswps.tile([128, CHUNK], F32, tag="usw_ps",
                                        name="usw_ps")
                    nc.tensor.matmul(sbc_ps[:, :], ones128[:, :], recb[:, :],
                                     start=True, stop=True)
                    nc.vector.tensor_copy(sbc[:, :], sbc_ps[:, :])

                def k_final(d):
                    ur = u_tiles[("k", d)]
                    kr = p1sb.tile([128, CHUNK], BF16, tag="krope", name="krope")
                    nc.vector.tensor_tensor(kr[:, :], ur[:, :], sk_bc[:, :],
                                            mybir.AluOpType.mult)
                    g, dd = d // HH, d % HH
                    nc.scalar.dma_start(
                        out=k_in[g].ap().rearrange("(p h t) -> p h t",
                                                   p=128, h=HH)[:, dd, :],
                        in_=kr[:, :])

                def q_final(d):
                    nc.vector.tensor_tensor(qT_sb[:, d * CHUNK:(d + 1) * CHUNK],
                                            u_tiles[("q", d)][:, :], sq_bc[:, :],
                                            mybir.AluOpType.mult)

                v_main = [v_in[g].ap()[0:VMAIN].rearrange(
                    "(p j x) -> p j x", p=128, j=3) for g in range(2)]
                v_rem = [v_in[g].ap()[VMAIN:VHALF].rearrange(
                    "(t x) -> t x", t=6) for g in range(2)]

                def v_proj(g):
                    # 512-col group g -> route into the two 768-col halves
                    for (t0, tsz) in TOKSUBS:
                        ps = vps.tile([128, 512], F32, tag="v_ps", name="v_ps")
                        for c in range(KC):
                            if t0 == 0:
                                wvc = p1wv.tile([128, 512], BF16, tag="wv_c",
                                                name="wv_c")
                                nc.sync.dma_start(out=wvc[:, :], in_=wv[g, c, :, :])
                                v_proj.wts[c] = wvc
                            nc.tensor.matmul(
                                ps[0:tsz, :],
                                xT_sb[:, c * CHUNK + t0:c * CHUNK + t0 + tsz],
                                v_proj.wts[c][:, :],
                                start=(c == 0), stop=(c == KC - 1))
                        vsb = p1sb.tile([128, 512], BF16, tag="vsb", name="vsb")
                        nc.vector.tensor_copy(vsb[0:tsz, :], ps[0:tsz, :])
                        c0 = 512 * g
                        off = 0
                        while off < 512:
                            half = (c0 + off) // 768
                            hcol = (c0 + off) % 768
                            n = min(512 - off, 768 - hcol)
                            if t0 < FULL:
                                nc.scalar.dma_start(
                                    out=v_main[half][:, t0 // 128, hcol:hcol + n],
                                    in_=vsb[0:tsz, off:off + n])
                            else:
                                nc.scalar.dma_start(
                                    out=v_rem[half][:, hcol:hcol + n],
                                    in_=vsb[0:tsz, off:off + n])
                            off += n
                v_proj.wts = {}

                def ag(in_t, out_t):
                    nc.gpsimd.collective_compute(
                        "AllGather", mybir.AluOpType.bypass,
                        ins=[in_t.ap().opt()],
                        outs=[out_t.ap().opt()],
                        replica_groups=[list(range(NC))],
                    )

                qk_proj("k", wk)
                qk_scales("k", sk_bc)
                for d in range(KC):
                    k_final(d)
                ag(k_in[0], k_out[0])
                v_proj(0)
                v_proj(1)  # completes v_in[0] (cols 0-767), starts v_in[1]
                ag(v_in[0], v_out[0])
                ag(k_in[1], k_out[1])
                v_proj(2)
                ag(v_in[1], v_out[1])

                quarters = {}

                def load_quarter(qi):
                    g, hs = qi // 2, QH * (qi % 2)
                    kt = akp.tile([128, NC, QH * CHUNK], BF16, tag="kt_q",
                                  name="kt_q")
                    kr = akrp.tile([128, QH, 48], BF16, tag="krem", name="krem")
                    vt = avp.tile([128, NC, 3, QH * 128], BF16, tag="vt_q",
                                  name="vt_q")
                    vr = avrp.tile([48, QH * 128], BF16, tag="vrem", name="vrem")
                    nc.gpsimd.dma_start(
                        out=kt[:, :, :],
                        in_=k_out[g].ap().rearrange("r (p x) -> p r x", p=128)
                        [:, :, hs * CHUNK:(hs + QH) * CHUNK])
                    krv = k_out[g].ap().rearrange("r (p h t) -> r p h t",
                                                  p=128, h=HH)
                    for r in range(NC):
                        nc.sync.dma_start(
                            out=kr[:, :, 6 * r:6 * r + 6],
                            in_=krv[r, :, hs:hs + QH, FULL:CHUNK])
                    vmv = v_out[g].ap()[:, 0:VMAIN].rearrange(
                        "r (p j x) -> r p j x", p=128, j=3)
                    vrv = v_out[g].ap()[:, VMAIN:VHALF].rearrange(
                        "r (t x) -> r t x", t=6)
                    for r in range(NC):
                        nc.scalar.dma_start(
                            out=vt[:, r, :, :],
                            in_=vmv[r, :, :, 128 * hs:128 * (hs + QH)])
                        nc.scalar.dma_start(
                            out=vr[6 * r:6 * r + 6, :],
                            in_=vrv[r, :, 128 * hs:128 * (hs + QH)])
                    quarters[qi] = (kt, kr, vt, vr)

                for qi in range(4):
                    load_quarter(qi)
                nc.sync.dma_start(out=wo_pre[:, :],
                                  in_=wo.ap().rearrange(
                                      "hh g p m -> g p hh m")[0, :, :, :])

                qk_proj("q", wq)
                qk_scales("q", sq_bc)
                for d in range(KC):
                    q_final(d)


            # ===== Phase 2: attention =====
            with tc.tile_pool(name="sc_ps", bufs=2, space="PSUM") as scps, \
                 tc.tile_pool(name="acc_ps", bufs=2, space="PSUM") as accps:

                finalize_prev = [lambda: None]

                def attn_head(h):
                    hl = h % QH
                    kt, krem, vt, vr = quarters[h // QH]
                    qh = qT_sb[:, h * CHUNK:(h + 1) * CHUNK]

                    acc = accps.tile([128, CHUNK], F32, tag="acc", name="acc")
                    sumacc = asum.tile([128, CHUNK], BF16, tag="sumacc",
                                       name="sumacc")
                    pr_t = []

                    def qk_group(gi):
                        sc = scps.tile([128, 3, 512], F32, tag="sc3", name="sc3")
                        pr = app.tile([128, 3, CHUNK], BF16, tag="pr3", name="pr3")
                        if gi < 8:
                            for j in range(3):
                                nc.tensor.matmul(
                                    sc[:, j, 0:CHUNK],
                                    kt[:, gi, hl * CHUNK + 128 * j:
                                       hl * CHUNK + 128 * j + 128],
                                    qh, start=True, stop=True)
                            nc.scalar.activation(
                                pr[:, :, :], sc[:, :, 0:CHUNK],
                                mybir.ActivationFunctionType.Exp,
                                bias=masks_sb[:, gi:gi + 1], scale=SCALE)
                        else:
                            nc.tensor.matmul(
                                sc[0:48, 0, 0:CHUNK],
                                krem[:, hl, :],
                                qh, start=True, stop=True)
                            nc.scalar.activation(
                                pr[0:48, 0, :], sc[0:48, 0, 0:CHUNK],
                                mybir.ActivationFunctionType.Exp,
                                bias=maskr_sb[0:48, 0:1], scale=SCALE)
                        pr_t.append(pr)

                    def pv_group(gi):
                        pr = pr_t[gi]
                        if gi < 8:
                            for j in range(3):
                                nc.tensor.matmul(
                                    acc[:, :],
                                    vt[:, gi, j, hl * 128:hl * 128 + 128],
                                    pr[:, j, :],
                                    start=(gi == 0 and j == 0), stop=False)
                                if gi == 0 and j == 0:
                                    nc.vector.tensor_copy(sumacc[:, :],
                                                          pr[:, j, :])
                                else:
                                    nc.vector.tensor_tensor(
                                        sumacc[:, :], sumacc[:, :], pr[:, j, :],
                                        mybir.AluOpType.add)
                        else:
                            nc.tensor.matmul(
                                acc[:, :], vr[0:48, hl * 128:hl * 128 + 128],
                                pr[0:48, 0, :], start=False, stop=True)
                            nc.vector.tensor_tensor(
                                sumacc[0:48, :], sumacc[0:48, :], pr[0:48, 0, :],
                                mybir.AluOpType.add)

                    qk_group(0)
                    qk_group(1)
                    finalize_prev[0]()          # prev head's sums + normalize
                    for gi in range(9):
                        if gi < 7:
                            qk_group(gi + 2)
                        pv_group(gi)

                    def finalize():
                        sums = scps.tile([128, 3, 512], F32, tag="sc3", name="sc3")
                        nc.tensor.matmul(sums[0:1, 0, 0:CHUNK], ones_bf[:, :],
                                         sumacc[:, :], start=True, stop=True)
                        rec = asb.tile([1, CHUNK], F32, tag="rec", name="rec")
                        rec16 = asb.tile([1, CHUNK], BF16, tag="rec16",
                                         name="rec16")
                        recb = asb.tile([128, CHUNK], BF16, tag="recb",
                                        name="recb")
                        nc.vector.reciprocal_approx_fast(rec[:, :],
                                                         sums[0:1, 0, 0:CHUNK])
                        nc.vector.tensor_copy(rec16[:, :], rec[:, :])
                        # broadcast 1/sums to all partitions on the PE
                        nc.tensor.matmul(sums[:, 1, 0:CHUNK], ones128[:, :],
                                         rec16[:, :], start=True, stop=True)
                        nc.vector.tensor_copy(recb[:, :], sums[:, 1, 0:CHUNK])
                        nc.vector.tensor_tensor(
                            attnT_sb[:, h * CHUNK:(h + 1) * CHUNK],
                            acc[:, :], recb[:, :], mybir.AluOpType.mult)

                    finalize_prev[0] = finalize

                for h in range(H):
                    attn_head(h)
                finalize_prev[0]()

            # ===== Phase 3: o-projection =====
            with tc.tile_pool(name="p4wb", bufs=4) as p4wb, \
                 tc.tile_pool(name="p4sb", bufs=3) as p4sb, \
                 tc.tile_pool(name="p4ps", bufs=2, space="PSUM") as p4ps:
                for gr in range(3):
                    pss = [p4ps.tile([128, 512], F32, tag=f"o_ps{s}",
                                     name=f"o_ps{s}") for s in range(4)]
                    for hh in range(KC):
                        if gr == 0:
                            wt = wo_pre[:, hh * 512:(hh + 1) * 512]
                        else:
                            wtt = p4wb.tile([128, 512], BF16, tag="wo_t",
                                            name="wo_t")
                            eng = nc.sync if hh % 2 == 0 else nc.scalar
                            eng.dma_start(out=wtt[:, :], in_=wo[hh, gr, :, :])
                            wt = wtt[:, :]
                        for s, (t0, tsz) in enumerate(TOKSUBS):
                            nc.tensor.matmul(
                                pss[s][0:tsz, :],
                                attnT_sb[:, hh * CHUNK + t0:hh * CHUNK + t0 + tsz],
                                wt,
                                start=(hh == 0), stop=(hh == KC - 1))
                    for s, (t0, tsz) in enumerate(TOKSUBS):
                        osb = p4sb.tile([128, 512], F32, tag="osb", name="osb")
                        nc.vector.tensor_copy(osb[0:tsz, :], pss[s][0:tsz, :])
                        nc.sync.dma_start(
                            out=out_part[t0:t0 + tsz, gr * 512:gr * 512 + 512],
                            in_=osb[0:tsz, :])

    nc.compile()
    return nc


_NC_CACHE = {}


def _get_nc():
    if "nc" not in _NC_CACHE:
        _NC_CACHE["nc"] = build_kernel()
    return _NC_CACHE["nc"]


def _prep_inputs(x, freqs_cos, freqs_sin, Wq, bq, Wk, bk, Wv, bv, Wo, bo,
                 gq, gk, frame_seqlen):
    assert int(frame_seqlen) == L
    assert np.all(np.asarray(bq) == 0) and np.all(np.asarray(bk) == 0)
    assert np.all(np.asarray(bv) == 0) and np.all(np.asarray(bo) == 0)
    assert np.all(np.asarray(gq) == 1) and np.all(np.asarray(gk) == 1)
    x2d = np.asarray(x, np.float32).reshape(T, D)
    xT_full = np.ascontiguousarray(x2d.T)

    perm = np.concatenate([
        np.concatenate([np.arange(0, 128, 2), np.arange(1, 128, 2)]) + 128 * h
        for h in range(H)])
    Wqp = np.asarray(Wq, np.float32)[:, perm]
    Wkp = np.asarray(Wk, np.float32)[:, perm]

    cosT = np.asarray(freqs_cos, np.float32).T
    sinT = np.asarray(freqs_sin, np.float32).T
    costab = np.concatenate([cosT, cosT], 0)
    sintab = np.concatenate([-sinT, sinT], 0)

    bf16 = ml_dtypes.bfloat16

    def tile_lhsT(w):  # [D, D] -> [KC, 128, KC*128]
        return np.ascontiguousarray(
            w.reshape(KC, 128, KC, 128).transpose(2, 1, 0, 3)
            .reshape(KC, 128, KC * 128))

    # partition-swap permutation matrix: out[m] = in[(m+64)%128]
    pm = np.zeros((128, 128), np.float32)
    for m in range(128):
        pm[(m + 64) % 128, m] = 1.0

    shared = {
        "wq": tile_lhsT(Wqp).astype(bf16), "wk": tile_lhsT(Wkp).astype(bf16),
        # wv: [D, D] -> [3, KC, 128, 512]
        "wv": np.ascontiguousarray(
            np.asarray(Wv, np.float32).reshape(KC, 128, 3, 512)
            .transpose(2, 0, 1, 3)).astype(bf16),
        # wo: [D, D] -> [KC, 3, 128, 512]
        "wo": np.ascontiguousarray(
            np.asarray(Wo, np.float32).reshape(KC, 128, 3, 512)
            .transpose(0, 2, 1, 3)).astype(bf16),
        "permat": pm.astype(bf16),
    }

    # remainder-tile mask: partition p (p<48) holds key 390*(p//6)+384+(p%6)
    in_maps = []
    for c in range(NC):
        t0 = c * CHUNK
        f_c = c // 2
        rank_frames = np.arange(NC) // 2
        mrank = np.where(rank_frames <= f_c, 0.0, NEG).astype(np.float32)
        mpad = np.broadcast_to(mrank, (128, NC)).copy()
        mr = np.full((128, 1), NEG, np.float32)
        for p in range(48):
            mr[p, 0] = mrank[p // 6]
        in_maps.append({
            **shared,
            "xT": np.ascontiguousarray(xT_full[:, t0:t0 + CHUNK]).astype(bf16),
            "cost": np.ascontiguousarray(costab[:, t0:t0 + CHUNK]).astype(bf16),
            "sint": np.ascontiguousarray(sintab[:, t0:t0 + CHUNK]).astype(bf16),
            "maskv": mpad,
            "maskr": mr,
        })
    return in_maps


def kernel(x, freqs_cos, freqs_sin, Wq, bq, Wk, bk, Wv, bv, Wo, bo,
           gq, gk, frame_seqlen):
    in_maps = _prep_inputs(x, freqs_cos, freqs_sin, Wq, bq, Wk, bk,
                           Wv, bv, Wo, bo, gq, gk, frame_seqlen)
    nc = _get_nc()
    res = run_bass_kernel_spmd(nc, in_maps, core_ids=list(range(NC)))
    out = np.empty((1, T, D), np.float32)
    for c in range(NC):
        out[0, c * CHUNK:(c + 1) * CHUNK, :] = res.results[c]["out_part"]
    return out


# revision 29
# speedup vs baseline: 1.1148x; 1.1148x over previous
"""Trainium2 Bass kernel for nn_CausalWanModel (frame-block-causal attention).

Self-contained: hardcodes shapes from the problem spec.
  B=1, T=3120, D=1536, H=12 heads, hd=128, frame_seqlen=780, 8 cores.

Sharding: sequence-parallel. Core c owns tokens [390c, 390c+390).
Per-core pipeline (emission order == engine queue order):
  k-proj (rope rotation fused; partition swap via a PE permutation matmul)
  -> rmsnorm scales -> k payload writes -> AllGather k half 0 trigger
  -> v-proj groups 0,1 -> AllGather v half 0 -> AllGather k half 1
  -> v group 2 -> AllGather v half 1 -> q-proj (overlaps the collectives).
  Attention per head over 8 rank-blocks of 3 full 128-key tiles plus one
  48-key remainder tile; block-causal mask as per-rank additive exp bias.
  K/V land in SBUF in 3-head quarters via dedicated pools (gpsimd/scalar
  queues) so loads start the moment each collective completes. Softmax
  denominators accumulate in bf16 on the Vector engine, ones-matmul total,
  reciprocal_approx_fast; head finalization is emitted after the next
  head's first QK groups so the PE never stalls on it.
  o-proj (column group 0 weights prefetched during attention).
"""

import math

import numpy as np
import ml_dtypes

import concourse.bacc as bacc
import concourse.mybir as mybir
import concourse.tile as tile
from concourse.bass_utils import run_bass_kernel_spmd

F32 = mybir.dt.float32
BF16 = mybir.dt.bfloat16

NC = 8
T = 3120
D = 1536
H = 12
HD = 128
L = 780  # frame_seqlen
CHUNK = T // NC  # 390 tokens per core
KC = D // 128  # 12 contraction chunks
EPS = 1e-6
SCALE = 1.0 / math.sqrt(HD)
NEG = -30000.0

FULL = 384            # 3 full key tiles per rank; 6 remainder keys
HH = 6                # heads per collective half
QH = 3                # heads per SBUF load quarter
KHALF = 128 * HH * CHUNK       # k payload elems per half, layout [p, h, t]
VMAIN = 128 * 3 * (HH * 128)   # v payload main part, layout [p, j, hc]
VHALF = VMAIN + 6 * (HH * 128)  # + remainder rows tail [t, hc]
TOKSUBS = ((0, 128), (128, 128), (256, 128), (384, 6))


def build_kernel(debug=False):
    nc = bacc.Bacc("TRN2", target_bir_lowering=False, debug=False, num_devices=NC)

    # ---- I/O ----
    xT = nc.dram_tensor("xT", [D, CHUNK], BF16, kind="ExternalInput")
    wq = nc.dram_tensor("wq", [KC, 128, KC * 128], BF16, kind="ExternalInput")
    wk = nc.dram_tensor("wk", [KC, 128, KC * 128], BF16, kind="ExternalInput")
    wv = nc.dram_tensor("wv", [3, KC, 128, 512], BF16, kind="ExternalInput")
    wo = nc.dram_tensor("wo", [KC, 3, 128, 512], BF16, kind="ExternalInput")
    permat = nc.dram_tensor("permat", [128, 128], BF16, kind="ExternalInput")
    cost = nc.dram_tensor("cost", [128, CHUNK], BF16, kind="ExternalInput")
    sint = nc.dram_tensor("sint", [128, CHUNK], BF16, kind="ExternalInput")
    maskv = nc.dram_tensor("maskv", [128, NC], F32, kind="ExternalInput")
    maskr = nc.dram_tensor("maskr", [128, 1], F32, kind="ExternalInput")
    out_part = nc.dram_tensor("out_part", [CHUNK, D], F32, kind="ExternalOutput")

    # ---- collective buffers ----
    k_in = [nc.dram_tensor(f"k_in{g}", [KHALF], BF16) for g in range(2)]
    v_in = [nc.dram_tensor(f"v_in{g}", [VHALF], BF16) for g in range(2)]
    k_out = [nc.dram_tensor(f"k_out{g}", [NC, KHALF], BF16, addr_space="Shared")
             for g in range(2)]
    v_out = [nc.dram_tensor(f"v_out{g}", [NC, VHALF], BF16, addr_space="Shared")
             for g in range(2)]

    with tile.TileContext(nc) as tc:
        with tc.tile_pool(name="const", bufs=1) as cpool, \
             tc.tile_pool(name="a_k", bufs=2) as akp, \
             tc.tile_pool(name="a_v", bufs=2) as avp, \
             tc.tile_pool(name="a_kr", bufs=2) as akrp, \
             tc.tile_pool(name="a_vr", bufs=2) as avrp, \
             tc.tile_pool(name="a_pr", bufs=6) as app, \
             tc.tile_pool(name="a_sb", bufs=3) as asb, \
             tc.tile_pool(name="a_sum", bufs=2) as asum, \
             tc.tile_pool(name="p4w", bufs=1) as p4w:
            qT_sb = cpool.tile([128, KC * CHUNK], BF16, tag="qT_sb")
            attnT_sb = cpool.tile([128, KC * CHUNK], BF16, tag="attnT_sb")
            cost_sb = cpool.tile([128, CHUNK], BF16, tag="cost_sb")
            sint_sb = cpool.tile([128, CHUNK], BF16, tag="sint_sb")
            masks_sb = cpool.tile([128, NC], F32, tag="masks_sb")
            maskr_sb = cpool.tile([128, 1], F32, tag="maskr_sb")
            perm_sb = cpool.tile([128, 128], BF16, tag="perm_sb")
            ones_bf = cpool.tile([128, 1], BF16, tag="ones_bf")
            ones128 = cpool.tile([1, 128], BF16, tag="ones128")
            eps_sb = cpool.tile([1, 1], F32, tag="eps_sb")
            sq_bc = cpool.tile([128, CHUNK], BF16, tag="sq_bc")
            sk_bc = cpool.tile([128, CHUNK], BF16, tag="sk_bc")

            nc.gpsimd.memset(ones_bf[:, :], 1.0)
            nc.gpsimd.memset(ones128[:, :], 1.0)
            nc.gpsimd.memset(eps_sb[:, :], EPS)
            nc.scalar.dma_start(out=cost_sb[:, :], in_=cost[:, :])
            nc.scalar.dma_start(out=sint_sb[:, :], in_=sint[:, :])
            nc.scalar.dma_start(out=masks_sb[:, :], in_=maskv[:, :])
            nc.scalar.dma_start(out=maskr_sb[:, :], in_=maskr[:, :])
            nc.scalar.dma_start(out=perm_sb[:, :], in_=permat[:, :])

            wo_pre = p4w.tile([128, KC * 512], BF16, tag="wo_pre")

            # ===== Phase 1: projections + rmsnorm + rope =====
            with tc.tile_pool(name="p1sb", bufs=3) as p1sb, \
                 tc.tile_pool(name="p1w", bufs=3) as p1w, \
                 tc.tile_pool(name="p1wv", bufs=12) as p1wv, \
                 tc.tile_pool(name="upool", bufs=1) as upool, \
                 tc.tile_pool(name="xpool", bufs=1) as xpool, \
                 tc.tile_pool(name="scl", bufs=2) as sclp, \
                 tc.tile_pool(name="qk_ps", bufs=2, space="PSUM") as qkps, \
                 tc.tile_pool(name="usw_ps", bufs=2, space="PSUM") as uswps, \
                 tc.tile_pool(name="v_ps", bufs=2, space="PSUM") as vps, \
                 tc.tile_pool(name="ssq_ps", bufs=1, space="PSUM") as ssqps:

                xT_sb = xpool.tile([128, KC * CHUNK], BF16, tag="xT_sb")
                for d in range(KC):
                    nc.scalar.dma_start(out=xT_sb[:, d * CHUNK:(d + 1) * CHUNK],
                                        in_=xT[d * 128:(d + 1) * 128, :])

                u_tiles = {(n, d): upool.tile([128, CHUNK], BF16,
                                              name=f"u_{n}_{d}", tag=f"u_{n}_{d}")
                           for n in ("q", "k") for d in range(KC)}
                ssq_ps = {}

                def qk_proj(name, w):
                    # projection + rmsnorm squares + rope rotation.
                    # ssq/perm matmuls for chunk d are emitted after chunk
                    # d+1's projection so the PE never waits on ACT/DVE.
                    ssq_ps[name] = ssqps.tile([1, CHUNK], F32, name=f"ssq_{name}",
                                              tag=f"ssq_{name}")
                    pend = []

                    def tail(d):
                        sq, usw = pend.pop(0)
                        nc.tensor.matmul(ssq_ps[name][:, :], ones_bf[:, :],
                                         sq[:, :],
                                         start=(d == 0), stop=(d == KC - 1))
                        ur = u_tiles[(name, d)]
                        nc.tensor.matmul(usw[:, :], perm_sb[:, :], ur[:, :],
                                         start=True, stop=True)
                        t1 = p1sb.tile([128, CHUNK], BF16, tag="rope_t1",
                                       name="rope_t1")
                        t2 = p1sb.tile([128, CHUNK], BF16, tag="rope_t2",
                                       name="rope_t2")
                        nc.vector.tensor_tensor(t1[:, :], ur[:, :], cost_sb[:, :],
                                                mybir.AluOpType.mult)
                        nc.vector.tensor_tensor(t2[:, :], usw[:, :], sint_sb[:, :],
                                                mybir.AluOpType.mult)
                        nc.vector.tensor_tensor(ur[:, :], t1[:, :], t2[:, :],
                                                mybir.AluOpType.add)

                    for d in range(KC):
                        wt = p1w.tile([128, D], BF16, tag="wqk_t", name="wqk_t")
                        nc.sync.dma_start(out=wt[:, :], in_=w[d, :, :])
                        ps = qkps.tile([128, CHUNK], F32, tag="proj_ps",
                                       name="proj_ps")
                        for c in range(KC):
                            nc.tensor.matmul(
                                ps[:, :],
                                wt[:, c * 128:(c + 1) * 128],
                                xT_sb[:, c * CHUNK:(c + 1) * CHUNK],
                                start=(c == 0), stop=(c == KC - 1))
                        ur = u_tiles[(name, d)]
                        nc.vector.tensor_copy(ur[:, :], ps[:, :])
                        sq = p1sb.tile([128, CHUNK], BF16, tag="sqsb", name="sqsb")
                        nc.scalar.activation(sq[:, :], ps[:, :],
                                             mybir.ActivationFunctionType.Square)
                        usw = uswps.tile([128, CHUNK], F32, tag="usw_ps",
                                         name="usw_ps")
                        pend.append((sq, usw))
                        if d > 0:
                            tail(d - 1)
                    tail(KC - 1)

                def qk_scales(name, sbc):
                    stile = sclp.tile([1, CHUNK], F32, tag="stile", name="stile")
                    rec = sclp.tile([1, CHUNK], F32, tag="rec", name="rec")
                    recb = sclp.tile([1, CHUNK], BF16, tag="recb16", name="recb16")
                    nc.scalar.activation(stile[:, :], ssq_ps[name][:, :],
                                         mybir.ActivationFunctionType.Sqrt,
                                         bias=eps_sb[:, :], scale=1.0 / D)
                    nc.vector.reciprocal_approx_fast(rec[:, :], stile[:, :])
                    nc.vector.tensor_copy(recb[:, :], rec[:, :])
                    # broadcast to all partitions via PE outer product
                    sbc_ps = uswps.tile([128, CHUNK], F32, tag="usw_ps",
                                        name="usw_ps")
                    nc.tensor.matmul(sbc_ps[:, :], ones128[:, :], recb[:, :],
                                     start=True, stop=True)
                    nc.vector.tensor_copy(sbc[:, :], sbc_ps[:, :])

                def k_final(d):
                    ur = u_tiles[("k", d)]
                    kr = p1sb.tile([128, CHUNK], BF16, tag="krope", name="krope")
                    nc.vector.tensor_tensor(kr[:, :], ur[:, :], sk_bc[:, :],
                                            mybir.AluOpType.mult)
                    g, dd = d // HH, d % HH
                    nc.scalar.dma_start(
                        out=k_in[g].ap().rearrange("(p h t) -> p h t",
                                                   p=128, h=HH)[:, dd, :],
                        in_=kr[:, :])

                def q_final(d):
                    nc.vector.tensor_tensor(qT_sb[:, d * CHUNK:(d + 1) * CHUNK],
                                            u_tiles[("q", d)][:, :], sq_bc[:, :],
                                            mybir.AluOpType.mult)

                v_main = [v_in[g].ap()[0:VMAIN].rearrange(
                    "(p j x) -> p j x", p=128, j=3) for g in range(2)]
                v_rem = [v_in[g].ap()[VMAIN:VHALF].rearrange(
                    "(t x) -> t x", t=6) for g in range(2)]

                def v_proj(g):
                    # 512-col group g -> route into the two 768-col halves
                    for (t0, tsz) in TOKSUBS:
                        ps = vps.tile([128, 512], F32, tag="v_ps", name="v_ps")
                        for c in range(KC):
                            if t0 == 0:
                                wvc = p1wv.tile([128, 512], BF16, tag="wv_c",
                                                name="wv_c")
                                nc.sync.dma_start(out=wvc[:, :], in_=wv[g, c, :, :])
                                v_proj.wts[c] = wvc
                            nc.tensor.matmul(
                                ps[0:tsz, :],
                                xT_sb[:, c * CHUNK + t0:c * CHUNK + t0 + tsz],
                                v_proj.wts[c][:, :],
                                start=(c == 0), stop=(c == KC - 1))
                        vsb = p1sb.tile([128, 512], BF16, tag="vsb", name="vsb")
                        nc.vector.tensor_copy(vsb[0:tsz, :], ps[0:tsz, :])
                        c0 = 512 * g
                        off = 0
                        while off < 512:
                            half = (c0 + off) // 768
                            hcol = (c0 + off) % 768
                            n = min(512 - off, 768 - hcol)
                            if t0 < FULL:
                                nc.scalar.dma_start(
                                    out=v_main[half][:, t0 // 128, hcol:hcol + n],
                                    in_=vsb[0:tsz, off:off + n])
                            else:
                                nc.scalar.dma_start(
                                    out=v_rem[half][:, hcol:hcol + n],
                                    in_=vsb[0:tsz, off:off + n])
                            off += n
                v_proj.wts = {}

                def ag(in_t, out_t):
                    nc.gpsimd.collective_compute(
                        "AllGather", mybir.AluOpType.bypass,
                        ins=[in_t.ap().opt()],
                        outs=[out_t.ap().opt()],
                        replica_groups=[list(range(NC))],
                    )

                qk_proj("k", wk)
                qk_scales("k", sk_bc)
                for d in range(KC):
                    k_final(d)
                ag(k_in[0], k_out[0])
                v_proj(0)
                v_proj(1)  # completes v_in[0] (cols 0-767), starts v_in[1]
                ag(v_in[0], v_out[0])
                ag(k_in[1], k_out[1])
                v_proj(2)
                ag(v_in[1], v_out[1])

                qk_proj("q", wq)
                qk_scales("q", sq_bc)
                for d in range(KC):
                    q_final(d)

                quarters = {}

                def load_quarter(qi):
                    g, hs = qi // 2, QH * (qi % 2)
                    kt = akp.tile([128, NC, QH * CHUNK], BF16, tag="kt_q",
                                  name="kt_q")
                    kr = akrp.tile([128, QH, 48], BF16, tag="krem", name="krem")
                    vt = avp.tile([128, NC, 3, QH * 128], BF16, tag="vt_q",
                                  name="vt_q")
                    vr = avrp.tile([48, QH * 128], BF16, tag="vrem", name="vrem")
                    nc.gpsimd.dma_start(
                        out=kt[:, :, :],
                        in_=k_out[g].ap().rearrange("r (p x) -> p r x", p=128)
                        [:, :, hs * CHUNK:(hs + QH) * CHUNK])
                    krv = k_out[g].ap().rearrange("r (p h t) -> r p h t",
                                                  p=128, h=HH)
                    for r in range(NC):
                        nc.sync.dma_start(
                            out=kr[:, :, 6 * r:6 * r + 6],
                            in_=krv[r, :, hs:hs + QH, FULL:CHUNK])
                    vmv = v_out[g].ap()[:, 0:VMAIN].rearrange(
                        "r (p j x) -> r p j x", p=128, j=3)
                    vrv = v_out[g].ap()[:, VMAIN:VHALF].rearrange(
                        "r (t x) -> r t x", t=6)
                    for r in range(NC):
                        nc.scalar.dma_start(
                            out=vt[:, r, :, :],
                            in_=vmv[r, :, :, 128 * hs:128 * (hs + QH)])
                        nc.scalar.dma_start(
                            out=vr[6 * r:6 * r + 6, :],
                            in_=vrv[r, :, 128 * hs:128 * (hs + QH)])
                    quarters[qi] = (kt, kr, vt, vr)

                for qi in range(4):
                    load_quarter(qi)
                nc.sync.dma_start(out=wo_pre[:, :],
                                  in_=wo.ap().rearrange(
                                      "hh g p m -> g p hh m")[0, :, :, :])



            # ===== Phase 2: attention =====
            with tc.tile_pool(name="sc_ps", bufs=2, space="PSUM") as scps, \
                 tc.tile_pool(name="acc_ps", bufs=2, space="PSUM") as accps:

                finalize_prev = [lambda: None]

                def attn_head(h):
                    hl = h % QH
                    kt, krem, vt, vr = quarters[h // QH]
                    qh = qT_sb[:, h * CHUNK:(h + 1) * CHUNK]

                    acc = accps.tile([128, CHUNK], F32, tag="acc", name="acc")
                    sumacc = asum.tile([128, CHUNK], BF16, tag="sumacc",
                                       name="sumacc")
                    pr_t = []

                    def qk_group(gi):
                        sc = scps.tile([128, 3, 512], F32, tag="sc3", name="sc3")
                        pr = app.tile([128, 3, CHUNK], BF16, tag="pr3", name="pr3")
                        if gi < 8:
                            for j in range(3):
                                nc.tensor.matmul(
                                    sc[:, j, 0:CHUNK],
                                    kt[:, gi, hl * CHUNK + 128 * j:
                                       hl * CHUNK + 128 * j + 128],
                                    qh, start=True, stop=True)
                            nc.scalar.activation(
                                pr[:, :, :], sc[:, :, 0:CHUNK],
                                mybir.ActivationFunctionType.Exp,
                                bias=masks_sb[:, gi:gi + 1], scale=SCALE)
                        else:
                            nc.tensor.matmul(
                                sc[0:48, 0, 0:CHUNK],
                                krem[:, hl, :],
                                qh, start=True, stop=True)
                            nc.scalar.activation(
                                pr[0:48, 0, :], sc[0:48, 0, 0:CHUNK],
                                mybir.ActivationFunctionType.Exp,
                                bias=maskr_sb[0:48, 0:1], scale=SCALE)
                        pr_t.append(pr)

                    def pv_group(gi):
                        pr = pr_t[gi]
                        if gi < 8:
                            for j in range(3):
                                nc.tensor.matmul(
                                    acc[:, :],
                                    vt[:, gi, j, hl * 128:hl * 128 + 128],
                                    pr[:, j, :],
                                    start=(gi == 0 and j == 0), stop=False)
                                if gi == 0 and j == 0:
                                    nc.vector.tensor_copy(sumacc[:, :],
                                                          pr[:, j, :])
                                else:
                                    nc.vector.tensor_tensor(
                                        sumacc[:, :], sumacc[:, :], pr[:, j, :],
                                        mybir.AluOpType.add)
                        else:
                            nc.tensor.matmul(
                                acc[:, :], vr[0:48, hl * 128:hl * 128 + 128],
                                pr[0:48, 0, :], start=False, stop=True)
                            nc.vector.tensor_tensor(
                                sumacc[0:48, :], sumacc[0:48, :], pr[0:48, 0, :],
                                mybir.AluOpType.add)

                    qk_group(0)
                    qk_group(1)
                    finalize_prev[0]()          # prev head's sums + normalize
                    for gi in range(9):
                        if gi < 7:
                            qk_group(gi + 2)
                        pv_group(gi)

                    def finalize():
                        sums = scps.tile([128, 3, 512], F32, tag="sc3", name="sc3")
                        nc.tensor.matmul(sums[0:1, 0, 0:CHUNK], ones_bf[:, :],
                                         sumacc[:, :], start=True, stop=True)
                        rec = asb.tile([1, CHUNK], F32, tag="rec", name="rec")
                        rec16 = asb.tile([1, CHUNK], BF16, tag="rec16",
                                         name="rec16")
                        recb = asb.tile([128, CHUNK], BF16, tag="recb",
                                        name="recb")
                        nc.vector.reciprocal_approx_fast(rec[:, :],
                                                         sums[0:1, 0, 0:CHUNK])
                        nc.vector.tensor_copy(rec16[:, :], rec[:, :])
                        # broadcast 1/sums to all partitions on the PE
                        nc.tensor.matmul(sums[:, 1, 0:CHUNK], ones128[:, :],
                                         rec16[:, :], start=True, stop=True)
                        nc.vector.tensor_copy(recb[:, :], sums[:, 1, 0:CHUNK])
                        nc.vector.tensor_tensor(
                            attnT_sb[:, h * CHUNK:(h + 1) * CHUNK],
                            acc[:, :], recb[:, :], mybir.AluOpType.mult)

                    finalize_prev[0] = finalize

                for h in range(H):
                    attn_head(h)
                finalize_prev[0]()

            # ===== Phase 3: o-projection =====
            with tc.tile_pool(name="p4wb", bufs=4) as p4wb, \
                 tc.tile_pool(name="p4sb", bufs=3) as p4sb, \
                 tc.tile_pool(name="p4ps", bufs=2, space="PSUM") as p4ps:
                for gr in range(3):
                    pss = [p4ps.tile([128, 512], F32, tag=f"o_ps{s}",
                                     name=f"o_ps{s}") for s in range(4)]
                    for hh in range(KC):
                        if gr == 0:
                            wt = wo_pre[:, hh * 512:(hh + 1) * 512]
                        else:
                            wtt = p4wb.tile([128, 512], BF16, tag="wo_t",
                                            name="wo_t")
                            eng = nc.sync if hh % 2 == 0 else nc.scalar
                            eng.dma_start(out=wtt[:, :], in_=wo[hh, gr, :, :])
                            wt = wtt[:, :]
                        for s, (t0, tsz) in enumerate(TOKSUBS):
                            nc.tensor.matmul(
                                pss[s][0:tsz, :],
                                attnT_sb[:, hh * CHUNK + t0:hh * CHUNK + t0 + tsz],
                                wt,
                                start=(hh == 0), stop=(hh == KC - 1))
                    for s, (t0, tsz) in enumerate(TOKSUBS):
                        osb = p4sb.tile([128, 512], F32, tag="osb", name="osb")
                        nc.vector.tensor_copy(osb[0:tsz, :], pss[s][0:tsz, :])
                        nc.sync.dma_start(
                            out=out_part[t0:t0 + tsz, gr * 512:gr * 512 + 512],
                            in_=osb[0:tsz, :])

    nc.compile()
    return nc


_NC_CACHE = {}


def _get_nc():
    if "nc" not in _NC_CACHE:
        _NC_CACHE["nc"] = build_kernel()
    return _NC_CACHE["nc"]


def _prep_inputs(x, freqs_cos, freqs_sin, Wq, bq, Wk, bk, Wv, bv, Wo, bo,
                 gq, gk, frame_seqlen):
    assert int(frame_seqlen) == L
    assert np.all(np.asarray(bq) == 0) and np.all(np.asarray(bk) == 0)
    assert np.all(np.asarray(bv) == 0) and np.all(np.asarray(bo) == 0)
    assert np.all(np.asarray(gq) == 1) and np.all(np.asarray(gk) == 1)
    x2d = np.asarray(x, np.float32).reshape(T, D)
    xT_full = np.ascontiguousarray(x2d.T)

    perm = np.concatenate([
        np.concatenate([np.arange(0, 128, 2), np.arange(1, 128, 2)]) + 128 * h
        for h in range(H)])
    Wqp = np.asarray(Wq, np.float32)[:, perm]
    Wkp = np.asarray(Wk, np.float32)[:, perm]

    cosT = np.asarray(freqs_cos, np.float32).T
    sinT = np.asarray(freqs_sin, np.float32).T
    costab = np.concatenate([cosT, cosT], 0)
    sintab = np.concatenate([-sinT, sinT], 0)

    bf16 = ml_dtypes.bfloat16

    def tile_lhsT(w):  # [D, D] -> [KC, 128, KC*128]
        return np.ascontiguousarray(
            w.reshape(KC, 128, KC, 128).transpose(2, 1, 0, 3)
            .reshape(KC, 128, KC * 128))

    # partition-swap permutation matrix: out[m] = in[(m+64)%128]
    pm = np.zeros((128, 128), np.float32)
    for m in range(128):
        pm[(m + 64) % 128, m] = 1.0

    shared = {
        "wq": tile_lhsT(Wqp).astype(bf16), "wk": tile_lhsT(Wkp).astype(bf16),
        # wv: [D, D] -> [3, KC, 128, 512]
        "wv": np.ascontiguousarray(
            np.asarray(Wv, np.float32).reshape(KC, 128, 3, 512)
            .transpose(2, 0, 1, 3)).astype(bf16),
        # wo: [D, D] -> [KC, 3, 128, 512]
        "wo": np.ascontiguousarray(
            np.asarray(Wo, np.float32).reshape(KC, 128, 3, 512)
            .transpose(0, 2, 1, 3)).astype(bf16),
        "permat": pm.astype(bf16),
    }

    # remainder-tile mask: partition p (p<48) holds key 390*(p//6)+384+(p%6)
    in_maps = []
    for c in range(NC):
        t0 = c * CHUNK
        f_c = c // 2
        rank_frames = np.arange(NC) // 2
        mrank = np.where(rank_frames <= f_c, 0.0, NEG).astype(np.float32)
        mpad = np.broadcast_to(mrank, (128, NC)).copy()
        mr = np.full((128, 1), NEG, np.float32)
        for p in range(48):
            mr[p, 0] = mrank[p // 6]
        in_maps.append({
            **shared,
            "xT": np.ascontiguousarray(xT_full[:, t0:t0 + CHUNK]).astype(bf16),
            "cost": np.ascontiguousarray(costab[:, t0:t0 + CHUNK]).astype(bf16),
            "sint": np.ascontiguousarray(sintab[:, t0:t0 + CHUNK]).astype(bf16),
            "maskv": mpad,
            "maskr": mr,
        })
    return in_maps


def kernel(x, freqs_cos, freqs_sin, Wq, bq, Wk, bk, Wv, bv, Wo, bo,
           gq, gk, frame_seqlen):
    in_maps = _prep_inputs(x, freqs_cos, freqs_sin, Wq, bq, Wk, bk,
                           Wv, bv, Wo, bo, gq, gk, frame_seqlen)
    nc = _get_nc()
    res = run_bass_kernel_spmd(nc, in_maps, core_ids=list(range(NC)))
    out = np.empty((1, T, D), np.float32)
    for c in range(NC):
        out[0, c * CHUNK:(c + 1) * CHUNK, :] = res.results[c]["out_part"]
    return out
